# revision 1
# baseline (speedup 1.0000x reference)
"""Multi-head attention (B=4, S=1024, DM=1024, H=16, D=64) on 8 Trainium2 cores.

Balanced head-sharding: core c owns global heads {2c, 2c+1} of ALL 4 batches,
so every core does exactly 1/8 of the total work regardless of how imbalanced
the per-batch Q_len/V_len are. The program processes 4 "slots", one per
batch, each compiled to that batch's own (QC_b, KC_b) = (ceil(Q_len_b/128),
ceil(min(Q_len_b, V_len_b)/128)) chunk counts (slots sorted by size so the
program is cacheable across runs with permuted batches; the host routes batch
data to slots). Masked work is skipped exactly: q >= 128*QC_b rows are zero
(host fills), keys >= 128*KC_b never attend; residual per-position masking is
data-driven (host zeroes V rows >= V_len and supplies a 0/1 "kones" column so
masked keys add 0 to the AV numerator and softmax denominator; the host
applies the query mask and the softmax division during reassembly).

All PE operands are bf16 (full rate at any free dim). Per slot: K/Q
projections accumulate over 8 dm chunks in PSUM and land in bf16 SBUF;
scores S^T[k,q] = KhT^T @ QhT with the slot's two heads row-packed at
partition offsets 0/64; P^T = Exp(S^T/8) on ACT; diagonal 128-blocks get a
0/1 triangle multiply; V-proj natural [k, hd] with the kones column appended
([Vh | kones] -> the AV matmul also produces denominators); O^T (+ denom
row) accumulates over k chunks in PSUM and is DMA'd out unnormalized.
Score chunks (gated on the slower ACT exp stream) are interleaved between
projection chunks and AV groups so the in-order PE queue never blocks long.

No collectives: host shards inputs and reassembles the output.
"""

import sys

if "/opt/trn_rl_repo" not in sys.path:
    sys.path.insert(0, "/opt/trn_rl_repo")

from contextlib import ExitStack

import numpy as np

import concourse.bacc as bacc
import concourse.tile as tile
from concourse import mybir

B, S, DM, H, D = 4, 1024, 1024, 16, 64
f32, bf16 = mybir.dt.float32, mybir.dt.bfloat16
BF16_NP = mybir.dt.np(bf16)
Exp = mybir.ActivationFunctionType.Exp

NH = 512  # union path: per-core output head-dims
_NC_CACHE = {}


def _plan_bal(Q_len, V_len):
    """Per-batch (QC_b, KC_b), sorted ascending by total size; returns
    (shapes, order) with order[s] = batch handled by slot s."""
    ql = [int(q) for q in np.ravel(Q_len)]
    vl = [int(v) for v in np.ravel(V_len)]
    shapes = []
    for b in range(B):
        qc = max(1, -(-ql[b] // 128))
        kc = max(1, -(-min(ql[b], vl[b]) // 128))
        shapes.append((qc, kc))
    order = sorted(range(B), key=lambda b: (shapes[b][0] + 2 * shapes[b][1], b))
    return tuple(shapes[b] for b in order), order


def _emit_bal(nc, tc, ctx, shapes, reps=1, paired=False):
    # paired: slots (0,1) and (2,3) are the SAME batch with different head
    # pairs (4 heads/core over 2 batches): X and kones are shared within a
    # slot pair (halving X DMA); W carries both head pairs.
    nslot = len(shapes)
    xslots = [0, 0, 2, 2] if paired else list(range(nslot))
    npair = 2 if paired else 1
    qcols = [128 * qc for qc, _ in shapes]
    kcols = [128 * kc for _, kc in shapes]
    qsl = [
        [(s, min(512, qcols[i] - s)) for s in range(0, qcols[i], 512)]
        for i in range(nslot)
    ]
    ksl = [
        [(s, min(512, kcols[i] - s)) for s in range(0, kcols[i], 512)]
        for i in range(nslot)
    ]

    _ld = sorted(set(xslots))
    Qd = {
        i: nc.dram_tensor(f"QT{i}", [128, 8, qcols[i]], bf16, kind="ExternalInput")
        for i in _ld
    }
    Kd = {
        i: nc.dram_tensor(f"KT{i}", [128, 8, kcols[i]], bf16, kind="ExternalInput")
        for i in _ld
    }
    Vd = {
        i: nc.dram_tensor(f"VT{i}", [128, 8, kcols[i]], bf16, kind="ExternalInput")
        for i in _ld
    }
    # this core's head pairs: K and Q weight slices stacked, then V
    Wkqd = nc.dram_tensor("Wkq", [128, npair, 2, 8, 128], bf16, kind="ExternalInput")
    Wvd = nc.dram_tensor("Wv", [128, npair, 8, 128], bf16, kind="ExternalInput")
    # loaded slots' kones concatenated: one tiny DMA, early (the VhO ones-col
    # copies sit at the head of the DVE queue and must not wait long)
    ld = sorted(set(xslots))  # slots whose X/kones are actually loaded
    kc_tot = sum(shapes[i][1] for i in ld)
    kod = nc.dram_tensor("kones", [128, kc_tot], bf16, kind="ExternalInput")
    # per slot: 130 rows, row 2*p+hx = head hx dim p (p=64 -> denominator);
    # bf16 out (host divides in f32 after upcast) halves output DMA bytes
    Od = [
        nc.dram_tensor(f"OT{i}", [130, qcols[i]], bf16, kind="ExternalOutput")
        for i in range(nslot)
    ]

    cons = ctx.enter_context(tc.tile_pool(name="cons", bufs=1))
    xt_pool = ctx.enter_context(tc.tile_pool(name="xt", bufs=1))
    wpool = ctx.enter_context(tc.tile_pool(name="w", bufs=1))
    qk_pool = ctx.enter_context(tc.tile_pool(name="qk", bufs=1))
    vh_pool = ctx.enter_context(tc.tile_pool(name="vh", bufs=1))
    n_pt = sum(
        min(shapes[i][1], (qs + qn) // 128)
        for i in range(nslot)
        for qs, qn in qsl[i]
    )
    # P tiles live from exp to their AV group; the interleave consumes a
    # group at most ~2 groups behind the score frontier, so a bounded pool
    # suffices (and keeps SBUF in range at large plans)
    pt_pool = ctx.enter_context(tc.tile_pool(name="pt", bufs=min(n_pt, 24)))
    osb_pool = ctx.enter_context(tc.tile_pool(name="osb", bufs=3))
    ps_pr = ctx.enter_context(tc.tile_pool(name="pspr", bufs=2, space="PSUM"))
    ps_sc = ctx.enter_context(tc.tile_pool(name="pssc", bufs=2, space="PSUM"))
    ps_av = ctx.enter_context(tc.tile_pool(name="psav", bufs=1, space="PSUM"))

    # tri01[p, t] = 1 if t >= p else 0 (diagonal causal mask, applied
    # multiplicatively to P = exp(S) after the exp)
    tri0 = cons.tile([128, 128], f32, tag="tri0")
    nc.vector.memset(tri0, 1.0)
    nc.gpsimd.affine_select(
        out=tri0,
        in_=tri0,
        compare_op=mybir.AluOpType.is_ge,
        fill=0.0,
        base=0,
        pattern=[[1, 128]],
        channel_multiplier=-1,
    )
    tri01 = cons.tile([128, 128], bf16, tag="tri01")
    nc.vector.tensor_copy(tri01, tri0)
    ko_all = cons.tile([128, kc_tot], bf16, tag="ko", name="ko_all")
    _off, _o = {}, 0
    for i in ld:
        _off[i] = _o
        _o += shapes[i][1]
    ko = [
        ko_all[:, _off[xslots[i]] : _off[xslots[i]] + shapes[i][1]]
        for i in range(nslot)
    ]

    def body(rep):
        # DMA plan: every DMA ultimately funnels through ONE shared transfer
        # engine (~345GB/s) — multiple queues add no bandwidth and let
        # late-needed transfers cut ahead of the critical path. So: a single
        # HWDGE queue (sync), one DMA per tensor, in strict consumption
        # order: wkq0, kones, xk0, xq0, wkq1, xk1, xq1, xv0, wv, xk2, xq2,
        # xv1, xk3, xq3, xv2, xv3 (matches the proj_rest emission order).
        # exact-shape tiles: a padded tile makes the DMA write strided
        # (256B runs -> half DMA bandwidth); contiguous DMAs merge to 2KB
        # runs at full rate. The dispatcher guards SBUF for large plans.
        wkq = wpool.tile([128, npair, 2, 8, 128], bf16, tag="wkq", name="wkq")
        wv = wpool.tile([128, npair, 8, 128], bf16, tag="wv", name="wv")
        xtk, xtq, xtv = [], [], []
        for i in range(nslot):
            if xslots[i] != i:  # shares the slot-pair leader's X
                xtk.append(xtk[xslots[i]])
                xtq.append(xtq[xslots[i]])
                xtv.append(xtv[xslots[i]])
                continue
            xtk.append(
                xt_pool.tile([128, 8, kcols[i]], bf16, tag=f"xtk{i}", name="xk")
            )
            xtq.append(
                xt_pool.tile([128, 8, qcols[i]], bf16, tag=f"xtq{i}", name="xq")
            )
            xtv.append(
                xt_pool.tile([128, 8, kcols[i]], bf16, tag=f"xtv{i}", name="xv")
            )
        nc.sync.dma_start(out=wkq[:, :, 0], in_=Wkqd.ap()[:, :, 0])
        if rep == 0:
            nc.sync.dma_start(out=ko_all, in_=kod.ap())
        def load_seq(xt, dram, cols):
            # big tensors in two sequential halves (same queue, same order):
            # the first projection matmuls start on the first half
            if cols > 384:
                for h in range(2):
                    nc.sync.dma_start(
                        out=xt[:, 4 * h : 4 * h + 4],
                        in_=dram.ap()[:, 4 * h : 4 * h + 4],
                    )
            else:
                nc.sync.dma_start(out=xt, in_=dram.ap())

        prev_v = None
        for i in ld:
            load_seq(xtk[i], Kd[i], kcols[i])
            if i == ld[0]:
                nc.sync.dma_start(out=wkq[:, :, 1], in_=Wkqd.ap()[:, :, 1])
            load_seq(xtq[i], Qd[i], qcols[i])
            if prev_v is not None:
                nc.sync.dma_start(out=xtv[prev_v], in_=Vd[prev_v].ap())
            if i == ld[min(1, len(ld) - 1)]:
                nc.sync.dma_start(out=wv, in_=Wvd.ap())
            prev_v = i
        nc.sync.dma_start(out=xtv[prev_v], in_=Vd[prev_v].ap())
        wk = [wkq[:, i % npair, 0] for i in range(nslot)]
        wq = [wkq[:, i % npair, 1] for i in range(nslot)]
        wvs = [wv[:, i % npair] for i in range(nslot)]

        QhT = [
            qk_pool.tile([128, qcols[i]], bf16, tag=f"qh{i}", name=f"QhT{i}")
            for i in range(nslot)
        ]
        KhT = [
            qk_pool.tile([128, kcols[i]], bf16, tag=f"kh{i}", name=f"KhT{i}")
            for i in range(nslot)
        ]
        VhO = [
            vh_pool.tile([128, shapes[i][1], 2, 65], bf16, tag=f"vh{i}", name=f"vh{i}")
            for i in range(nslot)
        ]
        for i in range(nslot):
            nc.vector.tensor_copy(
                out=VhO[i][:, :, :, 64:65],
                in_=ko[i][:, :, None, None].to_broadcast([128, shapes[i][1], 2, 1]),
            )

        def proj_slice(out_t, sl, xt, w, s_, n_):
            pm = ps_pr.tile([128, 512], f32, tag="pr", name="pm")
            for c in range(8):
                nc.tensor.matmul(
                    pm[:, :n_],
                    w[:, c],
                    xt[:, c, s_ : s_ + n_],
                    start=(c == 0),
                    stop=(c == 7),
                )
            nc.vector.tensor_copy(out=out_t[:, s_ : s_ + n_], in_=pm[:, :n_])

        def vproj_chunk(sl, kc):
            pm = ps_pr.tile([128, 512], f32, tag="pr", name="pmv")[:, :128]
            for c in range(8):
                nc.tensor.matmul(
                    pm,
                    xtv[sl][:, c, kc * 128 : (kc + 1) * 128],
                    wvs[sl][:, c],
                    start=(c == 0),
                    stop=(c == 7),
                )
            nc.vector.tensor_copy(
                out=VhO[sl][:, kc, :, 0:64],
                in_=pm.rearrange("p (h d) -> p h d", h=2),
            )

        PT = {}

        def score_chunk(sl, qs, qn, ki):
            off = max(0, ki * 128 - qs)
            kslice = slice(ki * 128, (ki + 1) * 128)
            st = ps_sc.tile([128, 2, 512], f32, tag="sc", name="st")
            for hx in range(2):
                hrow = slice(64 * hx, 64 * hx + 64)
                nc.tensor.matmul(
                    st[:, hx, off:qn],
                    KhT[sl][hrow, kslice],
                    QhT[sl][hrow, qs + off : qs + qn],
                    start=True,
                    stop=True,
                )
            pt = pt_pool.tile([128, 2, 512], bf16, tag="pt", name="pt")
            nc.scalar.activation(pt[:, :, off:qn], st[:, :, off:qn], Exp, scale=0.125)
            if ki * 128 >= qs:
                nc.vector.tensor_mul(
                    pt[:, :, off : off + 128],
                    pt[:, :, off : off + 128],
                    tri01[:, None, :].to_broadcast([128, 2, 128]),
                )
            PT[sl, qs, ki] = pt

        def av_group(sl, qs, qn, from_sc, act_helps):
            kmax = min(shapes[sl][1], (qs + qn) // 128)
            pool = ps_sc if from_sc else ps_av
            tag = "sc" if from_sc else "av"
            ot = pool.tile([128, 2, 512], f32, tag=tag, name="ot")
            for ki in range(kmax):
                off = max(0, ki * 128 - qs)
                pt = PT.pop((sl, qs, ki))
                for hx in range(2):
                    nc.tensor.matmul(
                        ot[:65, hx, off:qn],
                        VhO[sl][:, ki, hx, :],
                        pt[:, hx, off:qn],
                        start=(ki == 0),
                        stop=(ki == kmax - 1),
                    )
            osb = osb_pool.tile([65, 2, 512], bf16, tag="osb", name="osb")
            nc.vector.tensor_copy(out=osb[:, :, :qn], in_=ot[:65, :, :qn])
            nc.sync.dma_start(
                out=Od[sl].ap()[:, qs : qs + qn], in_=osb[:, :, :qn]
            )

        # --- interleaved emission (slots play the role head pairs did) ---
        def kq_items(sl):
            pairs = list(zip(ksl[sl], qsl[sl]))
            items = []
            for ks_, qs_ in pairs:
                items += [("K", sl) + ks_, ("Q", sl) + qs_]
            items += [("Q", sl) + s_ for s_ in qsl[sl][len(pairs) :]]
            return items

        proj_rest = kq_items(0)[2:]
        for i in range(1, nslot):
            ki_items = kq_items(i)
            if paired and i == 1:
                ki_items = ki_items[1:]  # K1 moves to the prologue (shares xtk0)
            proj_rest += ki_items + [("V", i - 1, kc) for kc in range(shapes[i - 1][1])]
        proj_rest += [("V", nslot - 1, kc) for kc in range(shapes[nslot - 1][1])]

        groups = [(i, qs, qn) for i in range(nslot) for qs, qn in qsl[i]]
        score_fifo = [
            (i, qs, qn, ki)
            for i, qs, qn in groups
            for ki in range(min(shapes[i][1], (qs + qn) // 128))
        ]
        kdone = [0] * nslot
        qdone = [set() for _ in range(nslot)]
        v_done = [0] * nslot

        def emit_proj(item):
            if item[0] == "K":
                _, sl, s_, n_ = item
                proj_slice(KhT[sl], sl, xtk[sl], wk[sl], s_, n_)
                kdone[sl] = s_ + n_
            elif item[0] == "Q":
                _, sl, s_, n_ = item
                proj_slice(QhT[sl], sl, xtq[sl], wq[sl], s_, n_)
                qdone[sl].add(s_)
            else:
                _, sl, kc = item
                vproj_chunk(sl, kc)
                v_done[sl] += 1

        emit_proj(kq_items(0)[0])
        if paired:
            emit_proj(kq_items(1)[0])  # shares xtk0: fills the xq0 DMA wait
        emit_proj(kq_items(0)[1])

        si = pi = gi = 0
        group_last_si = {}
        n_sc = len(score_fifo)

        def chunk_ready(sl, qs, qn, ki):
            return kdone[sl] >= (ki + 1) * 128 and qs in qdone[sl]

        while si < n_sc or gi < len(groups):
            progress = False
            for _ in range(2):
                if si < n_sc and chunk_ready(*score_fifo[si]):
                    sl, qs, qn, ki = score_fifo[si]
                    score_chunk(sl, qs, qn, ki)
                    si += 1
                    if ki == min(shapes[sl][1], (qs + qn) // 128) - 1:
                        group_last_si[len(group_last_si)] = si
                    progress = True
            if pi < len(proj_rest):
                emit_proj(proj_rest[pi])
                pi += 1
                progress = True
            if gi < len(groups):
                sl, qs, qn = groups[gi]
                kmax = min(shapes[sl][1], (qs + qn) // 128)
                scores_ok = (gi + 1 in group_last_si) or (
                    si >= n_sc and gi in group_last_si
                )
                if scores_ok and v_done[sl] >= kmax:
                    av_group(
                        sl,
                        qs,
                        qn,
                        from_sc=(si >= n_sc and gi % 2 == 0),
                        act_helps=(si >= n_sc and gi == len(groups) - 1),
                    )
                    gi += 1
                    progress = True
            assert progress, "interleave deadlock"

    for r in range(reps):
        body(r)


def _build_bal(shapes, reps=1, paired=False):
    nc = bacc.Bacc("TRN2", target_bir_lowering=False, debug=False)
    with tile.TileContext(nc) as tc, ExitStack() as ctx:
        _emit_bal(nc, tc, ctx, shapes, reps=reps, paired=paired)
    nc.compile()
    return nc


def get_nc_bal(reps=1, shapes=((8, 8),) * 4, paired=False):
    key = ('b', paired, tuple(shapes), reps)
    if key not in _NC_CACHE:
        _NC_CACHE[key] = _build_bal(tuple(shapes), reps=reps, paired=paired)
    return _NC_CACHE[key]


def make_in_maps_bal(Q_seq, K_seq, V_seq, WQ, WK, WV, Q_len, V_len):
    shapes, order = _plan_bal(Q_len, V_len)

    def pack(xt_2d):
        # [DM, n] -> [128, 8, n], row c*128+p -> [p, c]
        return np.ascontiguousarray(
            xt_2d.reshape(8, 128, xt_2d.shape[1]).transpose(1, 0, 2).astype(BF16_NP)
        )

    per_slot = []
    ko_parts = []
    for i, b in enumerate(order):
        qc, kc = shapes[i]
        qcols, kcols = 128 * qc, 128 * kc
        v = np.asarray(V_seq[b]).copy()
        v[int(V_len[b, 0]) :] = 0.0
        kones = (np.arange(kcols) < int(V_len[b, 0])).astype(np.float32)
        ko_parts.append(kones.reshape(kc, 128).T)
        per_slot.append(
            {
                f"QT{i}": pack(Q_seq[b].T[:, :qcols]),
                f"KT{i}": pack(K_seq[b].T[:, :kcols]),
                f"VT{i}": pack(v.T[:, :kcols]),
            }
        )
    ko_all = np.ascontiguousarray(
        np.concatenate(ko_parts, axis=1).astype(BF16_NP)
    )
    in_maps = []
    for c in range(8):
        cols = slice(c * 128, (c + 1) * 128)
        m = {
            "Wkq": np.ascontiguousarray(
                np.stack([pack(WK[:, cols]), pack(WQ[:, cols])], axis=1)
            ),
            "Wv": pack(WV[:, cols]),
            "kones": ko_all,
        }
        for d in per_slot:
            m.update(d)
        in_maps.append(m)
    return in_maps


def assemble_bal(results, Q_len, V_len):
    shapes, order = _plan_bal(Q_len, V_len)
    out = np.zeros((B, S, H * D), np.float32)
    for c in range(8):
        for i, b in enumerate(order):
            qcols = 128 * shapes[i][0]
            r = results[c][f"OT{i}"].reshape(65, 2, qcols).astype(np.float32)
            o = r[:64] / r[64:65]  # [64, 2, qcols]
            n = min(int(Q_len[b, 0]), qcols)
            # core c heads 2c, 2c+1 -> out cols [c*128, (c+1)*128)
            out[b, :n, c * 128 : (c + 1) * 128] = (
                o[:, :, :n].transpose(2, 1, 0).reshape(n, 128)
            )
    return out




def _plan_union(Q_len, V_len):
    ql = max(int(q) for q in np.ravel(Q_len))
    kl = max(min(int(q), int(v)) for q, v in zip(np.ravel(Q_len), np.ravel(V_len)))
    QC = max(1, -(-ql // 128))
    KC = max(1, -(-kl // 128))
    return QC, KC


def _emit_union(nc, tc, ctx, QC, KC, reps=1):
    Qcols, Kcols = 128 * QC, 128 * KC
    qslices = [(s, min(512, Qcols - s)) for s in range(0, Qcols, 512)]
    kslices = [(s, min(512, Kcols - s)) for s in range(0, Kcols, 512)]

    Qd = nc.dram_tensor("QT", [128, 8, Qcols], bf16, kind="ExternalInput")
    Kd = nc.dram_tensor("KT", [128, 8, Kcols], bf16, kind="ExternalInput")
    Vd = nc.dram_tensor("VT", [128, 8, Kcols], bf16, kind="ExternalInput")
    # Wk/Wq are stacked per head pair ([128, 4hp, 2kq, 8c, 128]) so one
    # 0.5MB DMA delivers both projections of a head pair; Wv keeps the flat
    # layout (its V-proj matmul streams all 512 head-dims of one dm chunk)
    Wkqd = nc.dram_tensor("Wkq", [128, 4, 2, 8, 128], bf16, kind="ExternalInput")
    Wvd = nc.dram_tensor("Wv", [128, 8, NH], bf16, kind="ExternalInput")
    kod = nc.dram_tensor("kones", [128, KC], bf16, kind="ExternalInput")
    # head-pair blocks of 130 rows; within a block, row 2*p+hx holds head
    # hx's O^T dim p (p=64 -> denominator), so one DMA covers both heads
    Od = nc.dram_tensor("OT", [65 * 8, Qcols], f32, kind="ExternalOutput")

    cons = ctx.enter_context(tc.tile_pool(name="cons", bufs=1))
    xt_pool = ctx.enter_context(tc.tile_pool(name="xt", bufs=1))
    wpool = ctx.enter_context(tc.tile_pool(name="w", bufs=1))
    qk_pool = ctx.enter_context(tc.tile_pool(name="qk", bufs=1))
    vh_pool = ctx.enter_context(tc.tile_pool(name="vh", bufs=1))
    n_pt = 4 * sum(min(KC, (qs + qn) // 128) for qs, qn in qslices)
    pt_pool = ctx.enter_context(tc.tile_pool(name="pt", bufs=n_pt))
    osb_pool = ctx.enter_context(tc.tile_pool(name="osb", bufs=3))
    ps_pr = ctx.enter_context(tc.tile_pool(name="pspr", bufs=2, space="PSUM"))
    ps_sc = ctx.enter_context(tc.tile_pool(name="pssc", bufs=2, space="PSUM"))
    ps_av = ctx.enter_context(tc.tile_pool(name="psav", bufs=1, space="PSUM"))

    # tri01[p, t] = 1 if t >= p else 0  (zeroes q < k on the diagonal block,
    # applied multiplicatively to P = exp(S) on gpsimd after the exp)
    tri0 = cons.tile([128, 128], f32, tag="tri0")
    nc.vector.memset(tri0, 1.0)
    nc.gpsimd.affine_select(
        out=tri0,
        in_=tri0,
        compare_op=mybir.AluOpType.is_ge,
        fill=0.0,
        base=0,
        pattern=[[1, 128]],
        channel_multiplier=-1,
    )
    tri01 = cons.tile([128, 128], bf16, tag="tri01")
    nc.vector.tensor_copy(tri01, tri0)
    ko = cons.tile([128, KC], bf16, tag="ko")

    def body(rep):
        # Queue plan: sync/scalar are HWDGE queues sharing one ~400GB/s
        # engine; gpsimd is SWDGE (own bandwidth, ~1us/DMA descriptor-gen on
        # Pool). Dispatch cost (~0.7-1.3us/DMA) dominates small transfers,
        # so chunk counts scale with size. xtk/xtq split scalar/gpsimd; W
        # stream on sync; xtv on scalar after xtq.
        wkq = wpool.tile([128, 4, 2, 8, 128], bf16, tag="wkq", name="wkq")
        xtk = xt_pool.tile([128, 8, Kcols], bf16, tag="xtk", name="xtk")
        xtq = xt_pool.tile([128, 8, Qcols], bf16, tag="xtq", name="xtq")

        def load_split(xt, dram):
            for h in range(4):
                eng = nc.scalar if h < 2 else nc.gpsimd
                eng.dma_start(
                    out=xt[:, 2 * h : 2 * h + 2],
                    in_=dram.ap()[:, 2 * h : 2 * h + 2],
                )

        nc.sync.dma_start(out=wkq[:, 0, 0], in_=Wkqd.ap()[:, 0, 0])
        load_split(xtk, Kd)
        nc.sync.dma_start(out=wkq[:, 0, 1], in_=Wkqd.ap()[:, 0, 1])
        load_split(xtq, Qd)
        for hp in range(1, 4):
            for kq in range(2):
                nc.sync.dma_start(out=wkq[:, hp, kq], in_=Wkqd.ap()[:, hp, kq])
        wv = wpool.tile([128, 8, NH], bf16, tag="wv", name="wv")
        for i in range(2):
            nc.sync.dma_start(out=wv[:, 4 * i : 4 * i + 4], in_=Wvd.ap()[:, 4 * i : 4 * i + 4])
        if rep == 0:
            nc.sync.dma_start(out=ko, in_=kod.ap())
        xtv = xt_pool.tile([128, 8, Kcols], bf16, tag="xtv", name="xtv")
        for i in range(2):
            nc.scalar.dma_start(
                out=xtv[:, 4 * i : 4 * i + 4], in_=Vd.ap()[:, 4 * i : 4 * i + 4]
            )
        wk = wkq[:, :, 0]
        wq = wkq[:, :, 1]

        QhT = qk_pool.tile([128, 4, Qcols], bf16, tag="qh", name="QhT")
        KhT = qk_pool.tile([128, 4, Kcols], bf16, tag="kh", name="KhT")
        VhO = vh_pool.tile([128, KC, 8, 65], bf16, tag="vh", name="vh")

        def proj_slice(out_t, hp, xt, w, s_, n_):
            # out_t[:, hp, s:s+n] = (W^T X^T) for the 128 head-dims of hp
            pm = ps_pr.tile([128, 512], f32, tag="pr", name="pm")
            for c in range(8):
                nc.tensor.matmul(
                    pm[:, :n_],
                    w[:, hp, c],
                    xt[:, c, s_ : s_ + n_],
                    start=(c == 0),
                    stop=(c == 7),
                )
            nc.vector.tensor_copy(out=out_t[:, hp, s_ : s_ + n_], in_=pm[:, :n_])

        def vproj_chunk(kc):
            # V natural [k, hd]: VhO[:, kc, h, 0:64]
            pm = ps_pr.tile([128, 512], f32, tag="pr", name="pmv")
            for c in range(8):
                nc.tensor.matmul(
                    pm,
                    xtv[:, c, kc * 128 : (kc + 1) * 128],
                    wv[:, c],
                    start=(c == 0),
                    stop=(c == 7),
                )
            nc.vector.tensor_copy(
                out=VhO[:, kc, :, 0:64],
                in_=pm.rearrange("p (h d) -> p h d", h=8),
            )

        PT = {}

        def score_chunk(hp, qs, qn, ki):
            off = max(0, ki * 128 - qs)
            ksl = slice(ki * 128, (ki + 1) * 128)
            st = ps_sc.tile([128, 2, 512], f32, tag="sc", name="st")
            for hx in range(2):
                hrow = slice(64 * hx, 64 * hx + 64)
                nc.tensor.matmul(
                    st[:, hx, off:qn],
                    KhT[hrow, hp, ksl],
                    QhT[hrow, hp, qs + off : qs + qn],
                    start=True,
                    stop=True,
                )
            pt = pt_pool.tile([128, 2, 512], bf16, tag="pt", name="pt")
            nc.scalar.activation(pt[:, :, off:qn], st[:, :, off:qn], Exp, scale=0.125)
            if ki * 128 >= qs:
                # DVE, not Pool: Pool's in-order queue carries the osb copies,
                # which must not delay later tri muls (exp -> tri -> AV path)
                nc.vector.tensor_mul(
                    pt[:, :, off : off + 128],
                    pt[:, :, off : off + 128],
                    tri01[:, None, :].to_broadcast([128, 2, 128]),
                )
            PT[hp, qs, ki] = pt

        def av_group(hp, qs, qn, from_sc, act_helps):
            # O^T (+ denominator row 64) = [Vh | kones]^T @ P^T
            # after the score stream drains, its PSUM pool is free: tail
            # groups rotate through it instead of the single "av" buffer
            kmax = min(KC, (qs + qn) // 128)
            pool = ps_sc if from_sc else ps_av
            tag = "sc" if from_sc else "av"
            ot = pool.tile([128, 2, 512], f32, tag=tag, name="ot")
            for ki in range(kmax):
                off = max(0, ki * 128 - qs)
                pt = PT.pop((hp, qs, ki))
                for hx in range(2):
                    nc.tensor.matmul(
                        ot[:65, hx, off:qn],
                        VhO[:, ki, 2 * hp + hx, :],
                        pt[:, hx, off:qn],
                        start=(ki == 0),
                        stop=(ki == kmax - 1),
                    )
            osb = osb_pool.tile([65, 2, 512], f32, tag="osb", name="osb")
            # single DVE copy: an ACT-assisted split costs more in dispatch
            # latency than it saves (measured on the bal/pair emitter)
            nc.vector.tensor_copy(out=osb[:, :, :qn], in_=ot[:65, :, :qn])
            nc.sync.dma_start(
                out=Od.ap()[130 * hp : 130 * hp + 130, qs : qs + qn],
                in_=osb[:, :, :qn],
            )

        nc.vector.tensor_copy(
            out=VhO[:, :, :, 64:65],
            in_=ko[:, :, None, None].to_broadcast([128, KC, 8, 1]),
        )

        # --- interleaved emission ---------------------------------------
        # PE executes in order, so score chunks (whose PSUM slots gate on the
        # slower ACT exp stream) are spread between projection chunks and AV
        # groups; AV(g) is emitted once group g+1's scores are all emitted
        # (its exps are then strictly older than the score-slot frontier).
        def kq_items(hp):
            # K slice a, Q slice a, K slice b, Q slice b: the low-q score
            # group of hp unblocks after just the two `a` slices
            pairs = list(zip(kslices, qslices))
            items = []
            for i, (ks_, qs_) in enumerate(pairs):
                items += [("K", hp) + ks_, ("Q", hp) + qs_]
            items += [("K", hp) + s_ for s_ in kslices[len(pairs) :]]
            items += [("Q", hp) + s_ for s_ in qslices[len(pairs) :]]
            return items

        proj_rest = (
            kq_items(0)[2:]
            + kq_items(1)
            + [("V", kc) for kc in range(KC)]
            + kq_items(2)
            + kq_items(3)
        )
        groups = [(hp, qs, qn) for hp in range(4) for qs, qn in qslices]
        score_fifo = [
            (hp, qs, qn, ki)
            for hp, qs, qn in groups
            for ki in range(min(KC, (qs + qn) // 128))
        ]
        kdone = [0, 0, 0, 0]  # K cols projected per hp
        qdone = [set() for _ in range(4)]  # Q slice starts projected per hp
        v_done = [0]  # V chunks projected

        def emit_proj(item):
            if item[0] == "K":
                _, hp, s_, n_ = item
                proj_slice(KhT, hp, xtk, wk, s_, n_)
                kdone[hp] = s_ + n_
            elif item[0] == "Q":
                _, hp, s_, n_ = item
                proj_slice(QhT, hp, xtq, wq, s_, n_)
                qdone[hp].add(s_)
            else:
                vproj_chunk(item[1])
                v_done[0] += 1

        emit_proj(kq_items(0)[0])
        emit_proj(kq_items(0)[1])

        si = 0  # scores emitted
        pi = 0  # proj items emitted
        gi = 0  # AV groups emitted
        group_last_si = {}  # group index -> si after its last score chunk
        n_sc = len(score_fifo)

        def chunk_ready(hp, qs, qn, ki):
            return kdone[hp] >= (ki + 1) * 128 and qs in qdone[hp]

        while si < n_sc or gi < len(groups):
            progress = False
            # up to two score chunks whose projection slices are emitted
            for _ in range(2):
                if si < n_sc and chunk_ready(*score_fifo[si]):
                    hp, qs, qn, ki = score_fifo[si]
                    score_chunk(hp, qs, qn, ki)
                    si += 1
                    if ki == min(KC, (qs + qn) // 128) - 1:
                        group_last_si[len(group_last_si)] = si
                    progress = True
            # one projection item
            if pi < len(proj_rest):
                emit_proj(proj_rest[pi])
                pi += 1
                progress = True
            # AV group g once group g+1's scores are done (or none left) and
            # its V chunks are projected
            if gi < len(groups):
                hp, qs, qn = groups[gi]
                kmax = min(KC, (qs + qn) // 128)
                scores_ok = (gi + 1 in group_last_si) or (
                    si >= n_sc and gi in group_last_si
                )
                if scores_ok and v_done[0] >= kmax:
                    av_group(
                        hp,
                        qs,
                        qn,
                        from_sc=(si >= n_sc and gi % 2 == 0),
                        act_helps=(si >= n_sc and gi == len(groups) - 1),
                    )
                    gi += 1
                    progress = True
            assert progress, "interleave deadlock"

    for r in range(reps):
        body(r)


def _build_union(QC, KC, reps=1):
    nc = bacc.Bacc("TRN2", target_bir_lowering=False, debug=False)
    with tile.TileContext(nc) as tc, ExitStack() as ctx:
        _emit_union(nc, tc, ctx, QC, KC, reps=reps)
    nc.compile()
    return nc


def get_nc_union(reps=1, plan=(8, 8)):
    key = (plan, reps)
    if key not in _NC_CACHE:
        _NC_CACHE[('u',) + key] = _build_union(*plan, reps=reps)
    return _NC_CACHE[('u',) + key]


def make_in_maps_union(Q_seq, K_seq, V_seq, WQ, WK, WV, Q_len, V_len):
    QC, KC = _plan_union(Q_len, V_len)
    Qcols, Kcols = 128 * QC, 128 * KC
    karange = np.arange(Kcols)

    def pack(xt_2d):
        # [DM, n] -> [128, 8, n], row c*128+p -> [p, c]
        return np.ascontiguousarray(
            xt_2d.reshape(8, 128, xt_2d.shape[1]).transpose(1, 0, 2).astype(BF16_NP)
        )

    def pack_hp(w_2d):
        # [128, 8c, 512] -> [128, 4hp, 8c, 128] (contiguous per-head-pair DMA)
        return np.ascontiguousarray(
            pack(w_2d).reshape(128, 8, 4, 128).transpose(0, 2, 1, 3)
        )

    qt = [pack(Q_seq[b].T[:, :Qcols]) for b in range(B)]
    kt = [pack(K_seq[b].T[:, :Kcols]) for b in range(B)]
    vt = []
    for b in range(B):
        v = np.asarray(V_seq[b]).copy()
        v[int(V_len[b, 0]) :] = 0.0  # masked keys contribute exactly 0
        vt.append(pack(v.T[:, :Kcols]))
    wkq = [
        np.ascontiguousarray(
            np.stack(
                [
                    pack_hp(WK[:, hh * NH : (hh + 1) * NH]),
                    pack_hp(WQ[:, hh * NH : (hh + 1) * NH]),
                ],
                axis=2,
            )
        )
        for hh in range(2)
    ]
    wv = [pack(WV[:, hh * NH : (hh + 1) * NH]) for hh in range(2)]
    in_maps = []
    for c in range(8):
        b, hh = c // 2, c % 2
        kones = (karange < int(V_len[b, 0])).astype(np.float32)
        in_maps.append(
            {
                "QT": qt[b],
                "KT": kt[b],
                "VT": vt[b],
                "Wkq": wkq[hh],
                "Wv": wv[hh],
                "kones": np.ascontiguousarray(
                    kones.reshape(KC, 128).T.astype(BF16_NP)
                ),
            }
        )
    return in_maps


def assemble_union(results, Q_len, plan):
    QC, _ = plan
    Qcols = 128 * QC
    out = np.zeros((B, S, H * D), np.float32)
    for c in range(8):
        b, hh = c // 2, c % 2
        # row layout: [4 hp, 65 p, 2 hx]; p=64 is the denominator row
        r = results[c]["OT"].reshape(4, 65, 2, Qcols)
        o = r[:, :64] / r[:, 64:65]  # [4, 64, 2, Qcols] normalized
        ql = int(Q_len[b, 0])
        n = min(ql, Qcols)
        # out col for head 2*hp+hx, dim d = (2*hp+hx)*64 + d
        out[b, :n, hh * NH : (hh + 1) * NH] = (
            o[:, :, :, :n].transpose(3, 0, 2, 1).reshape(n, NH)
        )
    return out




# --- mode selection: calibrated cost estimates (TimelineSim fits) ----------

def _est_union_ns(QC, KC):
    qcols = 128 * QC
    proj = 32 * 128 * (QC + 2 * KC)
    attn = 16 * sum(max(0, qcols - 128 * ki) for ki in range(KC))
    return 1.45 * 0.4167 * (proj + attn) + 2000


def _est_bal_ns(shapes):
    pe = 0
    cols = 0
    for qc, kc in shapes:
        qcols = 128 * qc
        pe += 8 * 128 * (qc + 2 * kc)
        pe += 4 * sum(max(0, qcols - 128 * ki) for ki in range(kc))
        cols += 128 * (qc + 2 * kc)
    return max(1.55 * 0.4167 * pe, 8.1 * cols) + 9000


def _plan_pair(Q_len, V_len):
    """Paired sharding: cores 0-3 take batches (B0, B2), cores 4-7 (B1, B3)
    of the size-sorted order, 4 heads each; program slots are the pairwise
    union shapes (u01, u01, u23, u23)."""
    shapes, order = _plan_bal(Q_len, V_len)
    u01 = (max(shapes[0][0], shapes[1][0]), max(shapes[0][1], shapes[1][1]))
    u23 = (max(shapes[2][0], shapes[3][0]), max(shapes[2][1], shapes[3][1]))
    return (u01, u01, u23, u23), order


def _est_pair_ns(shapes4):
    pe = 0
    for qc, kc in shapes4:
        qcols = 128 * qc
        pe += 8 * 128 * (qc + 2 * kc)
        pe += 4 * sum(max(0, qcols - 128 * ki) for ki in range(kc))
    cols = 128 * (
        shapes4[0][0] + 2 * shapes4[0][1] + shapes4[2][0] + 2 * shapes4[2][1]
    ) + 384  # + the extra 0.75MB of W vs 2-head mode
    return max(1.55 * 0.4167 * pe, 8.1 * cols) + 9000


def make_in_maps_pair(Q_seq, K_seq, V_seq, WQ, WK, WV, Q_len, V_len):
    shapes4, order = _plan_pair(Q_len, V_len)

    def pack(xt_2d):
        return np.ascontiguousarray(
            xt_2d.reshape(8, 128, xt_2d.shape[1]).transpose(1, 0, 2).astype(BF16_NP)
        )

    per_group = []
    for g in range(2):
        data = {}
        ko_parts = []
        for slot, b in ((0, order[g]), (2, order[2 + g])):
            qc, kc = shapes4[slot]
            qcols, kcols = 128 * qc, 128 * kc
            v = np.asarray(V_seq[b]).copy()
            v[int(V_len[b, 0]) :] = 0.0
            data[f"QT{slot}"] = pack(Q_seq[b].T[:, :qcols])
            data[f"KT{slot}"] = pack(K_seq[b].T[:, :kcols])
            data[f"VT{slot}"] = pack(v.T[:, :kcols])
            ko_parts.append(
                (np.arange(kcols) < int(V_len[b, 0]))
                .astype(np.float32)
                .reshape(kc, 128)
                .T
            )
        data["kones"] = np.ascontiguousarray(
            np.concatenate(ko_parts, axis=1).astype(BF16_NP)
        )
        per_group.append(data)
    in_maps = []
    for c in range(8):
        g, q = c // 4, c % 4
        wkq_p, wv_p = [], []
        for p in range(2):
            cols = slice(q * 256 + p * 128, q * 256 + (p + 1) * 128)
            wkq_p.append(np.stack([pack(WK[:, cols]), pack(WQ[:, cols])], axis=1))
            wv_p.append(pack(WV[:, cols]))
        m = {
            "Wkq": np.ascontiguousarray(np.stack(wkq_p, axis=1)),
            "Wv": np.ascontiguousarray(np.stack(wv_p, axis=1)),
        }
        m.update(per_group[g])
        in_maps.append(m)
    return in_maps


def assemble_pair(results, Q_len, V_len):
    shapes4, order = _plan_pair(Q_len, V_len)
    out = np.zeros((B, S, H * D), np.float32)
    for c in range(8):
        g, q = c // 4, c % 4
        for base, b in ((0, order[g]), (2, order[2 + g])):
            for p in range(2):
                i = base + p
                qcols = 128 * shapes4[i][0]
                r = (
                    results[c][f"OT{i}"]
                    .reshape(65, 2, qcols)
                    .astype(np.float32)
                )
                o = r[:64] / r[64:65]
                n = min(int(Q_len[b, 0]), qcols)
                out[b, :n, q * 256 + p * 128 : q * 256 + (p + 1) * 128] = (
                    o[:, :, :n].transpose(2, 1, 0).reshape(n, 128)
                )
    return out


def plan_from_lengths(Q_len, V_len):
    """Returns ('bal', shapes) or ('union', (QC, KC)) by estimated cost."""
    shapes, _ = _plan_bal(Q_len, V_len)
    shapes4, _ = _plan_pair(Q_len, V_len)
    union = _plan_union(Q_len, V_len)
    # resident-X SBUF guards (~2KB/partition per (qc + 2kc) unit)
    cands = [("union", union, _est_union_ns(*union))]
    if sum(qc + 2 * kc for qc, kc in shapes) <= 56:
        cands.append(("bal", shapes, _est_bal_ns(shapes)))
    if (
        shapes4[0][0] + 2 * shapes4[0][1] + shapes4[2][0] + 2 * shapes4[2][1]
        <= 56
    ):
        cands.append(("pair", shapes4, _est_pair_ns(shapes4)))
    mode, p, _ = min(cands, key=lambda t: t[2])
    return (mode, p)


def get_nc(reps=1, plan=("union", (8, 8))):
    mode, p = plan
    if mode == "bal":
        return get_nc_bal(reps=reps, shapes=p)
    if mode == "pair":
        return get_nc_bal(reps=reps, shapes=p, paired=True)
    return get_nc_union(reps=reps, plan=p)


def make_in_maps(Q_seq, K_seq, V_seq, WQ, WK, WV, Q_len, V_len):
    mode, _ = plan_from_lengths(Q_len, V_len)
    f = {
        "bal": make_in_maps_bal,
        "pair": make_in_maps_pair,
        "union": make_in_maps_union,
    }[mode]
    return f(Q_seq, K_seq, V_seq, WQ, WK, WV, Q_len, V_len)


def assemble(results, Q_len, V_len):
    mode, p = plan_from_lengths(Q_len, V_len)
    if mode == "bal":
        return assemble_bal(results, Q_len, V_len)
    if mode == "pair":
        return assemble_pair(results, Q_len, V_len)
    return assemble_union(results, Q_len, p)


def kernel(Q_seq, K_seq, V_seq, WQ, WK, WV, Q_len, V_len):
    from concourse.bass_utils import run_bass_kernel_spmd

    Q_seq, K_seq, V_seq = (np.asarray(x, np.float32) for x in (Q_seq, K_seq, V_seq))
    WQ, WK, WV = (np.asarray(x, np.float32) for x in (WQ, WK, WV))
    Q_len, V_len = np.asarray(Q_len), np.asarray(V_len)
    plan = plan_from_lengths(Q_len, V_len)
    nc = get_nc(plan=plan)
    in_maps = make_in_maps(Q_seq, K_seq, V_seq, WQ, WK, WV, Q_len, V_len)
    r = run_bass_kernel_spmd(nc, in_maps, core_ids=list(range(8)))
    return assemble(r.results, Q_len, V_len)

